# revision 5
# baseline (speedup 1.0000x reference)
"""DiffTexture bilinear sampling kernel for TRN2 (8 NeuronCores).

Strategy (data-parallel over sample points, texture replicated):
  - Each core handles N/8 = 1,048,576 points in 16 macro-tiles of
    128x512.
  - Phase 1 (per core): build a bf16 u-pair table in DRAM:
      B[u, v] = [T[u,v,:], T[u+1,v,:]]  (6 bf16 = 12B per entry)
    for u in [0, 2046], v in [0, 2047]. Reading entries (u,v0),(u,v0+1)
    as one 24B block yields all four texels of the bilinear footprint.
    Single-read build: each 128-row block is loaded from HBM once,
    converted to bf16, and the +1-row copy comes from a partition-
    shifted SBUF->SBUF DMA (plus one cast-DMA for the boundary row).
  - Phase 2: per 128-point chunk, one indirect DMA gathers each point's
    24B block (idx = u0*2048 + v0). The toolchain consumes exactly one
    index per partition per instruction, so the gather stream is
    8192 Pool instructions/core (~1.3us each) and dominates; everything
    else is arranged to hide under it. uvs are uniform in [-1, 1) so
    u = (x+1)*1023.5 < 2047: floor(u) <= 2046 and the +1 row/col is
    always in range -- no clamping or edge fix-ups needed.
  - Weights (faithful to the reference's quirk: the FLOOR row/col gets
    the frac weight a): weight of the +1 row/col = (1-fr)*[u != floor],
    carried negated: nw = (fr-1)*[u != floor]; blend via
    r = p_lo + nw*(p_lo - p_hi) using stride-0 broadcast APs for the
    channel dim; tanh on ACT.

Floor is built from the DVE round-to-nearest f32->i32 cast plus a
compare fix-up (no floor ALU op on TRN2).
"""

import numpy as np

import concourse.bass as bass
import concourse.bacc as bacc
import concourse.mybir as mybir
from concourse import tile
from concourse.bass_utils import run_bass_kernel_spmd

H = 2048
W = 2048
N_FULL = 8388608
NCORES = 8
P = 128
K = 512                  # points per partition per macro-tile
TPOINTS = P * K          # 65536 points per macro-tile

f32 = mybir.dt.float32
bf16 = mybir.dt.bfloat16
i32 = mybir.dt.int32

# Columns of tile 0 gathered straight from the f32 texture BEFORE the table
# barrier (2 instrs/column instead of 1, but they run while the Pool engine
# would otherwise idle during the table build).
C0 = 128

ROW = W * 3              # texture row, f32 elements (6144)
BROW = W * 6             # pair-table row, bf16 elements (12288)
BROWS = H - 1            # pair-table rows built (2047)


def _ap(t_ap, extra_offset, dims):
    """Build a raw AP on the same tensor as t_ap with given dims."""
    return bass.AP(t_ap.tensor, t_ap.offset + extra_offset, dims)


def build_nc(npc):
    """Build the per-core Bass program for npc points (npc % TPOINTS == 0)."""
    ntiles = npc // TPOINTS
    nc = bacc.Bacc("TRN2", target_bir_lowering=False)

    uvs = nc.dram_tensor("uvs", [npc, 2], f32, kind="ExternalInput")
    texture = nc.dram_tensor("texture", [H, W, 3], f32, kind="ExternalInput")
    out = nc.dram_tensor("out", [npc, 3], f32, kind="ExternalOutput")
    btab = nc.dram_tensor("btab", [BROWS * W, 6], bf16)  # internal, 50MB

    tex_flat = texture[:].rearrange("h w c -> (h w c)")
    tex_hw = texture[:].rearrange("h w c -> (h w) c")
    uvs_t = uvs[:].rearrange("(t p k) c -> t p (k c)", t=ntiles, p=P, k=K)
    out_t = out[:].rearrange("(t p k) c -> t p (k c)", t=ntiles, p=P, k=K)

    with tile.TileContext(nc) as tc:
        with tc.tile_pool(name="persist", bufs=1) as pp:

            def coord(pool, src_ap, name):
                # returns (nw = -(weight of +1 row/col), floor as f32,
                #          plus two scratch tiles now free for reuse)
                cu = pool.tile([P, K], f32, tag=f"{name}_cu")
                nc.vector.tensor_scalar(
                    out=cu[:], in0=src_ap, scalar1=1.0, scalar2=1023.5,
                    op0=mybir.AluOpType.add, op1=mybir.AluOpType.mult)
                ci = pool.tile([P, K], i32, tag=f"{name}_ci")
                nc.vector.tensor_copy(ci[:], cu[:])        # rint -> i32
                rcf = pool.tile([P, K], f32, tag=f"{name}_rcf")
                nc.vector.tensor_copy(rcf[:], ci[:])       # rint as f32
                d = pool.tile([P, K], f32, tag=f"{name}_d")
                nc.vector.tensor_tensor(                    # d = rcf - u
                    out=d[:], in0=rcf[:], in1=cu[:],
                    op=mybir.AluOpType.subtract)
                nc.vector.tensor_scalar(                    # d = (rcf > u)
                    out=d[:], in0=d[:], scalar1=0.0, scalar2=0.0,
                    op0=mybir.AluOpType.max, op1=mybir.AluOpType.not_equal)
                nc.vector.tensor_tensor(                    # rcf = floor
                    out=rcf[:], in0=rcf[:], in1=d[:],
                    op=mybir.AluOpType.subtract)
                fr = pool.tile([P, K], f32, tag=f"{name}_fr")
                nc.vector.tensor_tensor(                    # fr = u - floor
                    out=fr[:], in0=cu[:], in1=rcf[:],
                    op=mybir.AluOpType.subtract)
                nc.vector.tensor_tensor(                    # d = (u != floor)
                    out=d[:], in0=cu[:], in1=rcf[:],
                    op=mybir.AluOpType.not_equal)
                nc.vector.scalar_tensor_tensor(             # cu = (fr-1)*d
                    out=cu[:], in0=fr[:], scalar=1.0, in1=d[:],
                    op0=mybir.AluOpType.subtract,
                    op1=mybir.AluOpType.mult)
                return cu, rcf, fr, ci

            def head_tile(pool, ti):
                # uv -> coords -> idx (+ negated weights) for one tile
                uv = pool.tile([P, 2 * K], f32, tag="uv")
                nc.sync.dma_start(out=uv[:], in_=uvs_t[ti])
                x_ap = _ap(uv[:], 0, [uv[:].ap[0], [2, K]])
                y_ap = _ap(uv[:], 1, [uv[:].ap[0], [2, K]])
                nwu, u0f, fr_u, ci_u = coord(pool, x_ap, "u")
                nwv, v0f, _fv, _cv = coord(pool, y_ap, "v")
                idxf = fr_u
                nc.vector.scalar_tensor_tensor(
                    out=idxf[:], in0=u0f[:], scalar=float(W), in1=v0f[:],
                    op0=mybir.AluOpType.mult, op1=mybir.AluOpType.add)
                idx = ci_u
                nc.vector.tensor_copy(idx[:], idxf[:])
                return idx, nwu, nwv

            def blend(c0, n, p00, p01, p10, p11, nwv, nwu, r0, r1, res):
                # res[cols c0:c0+n] = bilinear blend; APs passed per source.
                # nwv/nwu are [P, K] tiles broadcast over channels with a
                # stride-0 inner dim.
                def v3(t):
                    return _ap(t[:], 3 * c0, [t[:].ap[0], [3, n], [1, 3]])

                def w3(t):
                    return _ap(t[:], c0, [t[:].ap[0], [1, n], [0, 3]])
                # r0 = p00 + nwv*(p00 - p01)   (within row u0)
                nc.vector.tensor_tensor(out=v3(r0), in0=p00, in1=p01,
                                        op=mybir.AluOpType.subtract)
                nc.vector.tensor_tensor(out=v3(r0), in0=v3(r0), in1=w3(nwv),
                                        op=mybir.AluOpType.mult)
                nc.vector.tensor_tensor(out=v3(r0), in0=v3(r0), in1=p00,
                                        op=mybir.AluOpType.add)
                # r1 = p10 + nwv*(p10 - p11)   (within row u0+1)
                nc.vector.tensor_tensor(out=v3(r1), in0=p10, in1=p11,
                                        op=mybir.AluOpType.subtract)
                nc.vector.tensor_tensor(out=v3(r1), in0=v3(r1), in1=w3(nwv),
                                        op=mybir.AluOpType.mult)
                nc.vector.tensor_tensor(out=v3(r1), in0=v3(r1), in1=p10,
                                        op=mybir.AluOpType.add)
                # res = r0 + nwu*(r0 - r1)
                nc.vector.tensor_tensor(out=v3(res), in0=v3(r0), in1=v3(r1),
                                        op=mybir.AluOpType.subtract)
                nc.vector.tensor_tensor(out=v3(res), in0=v3(res),
                                        in1=w3(nwu),
                                        op=mybir.AluOpType.mult)
                nc.vector.tensor_tensor(out=v3(res), in0=v3(res), in1=v3(r0),
                                        op=mybir.AluOpType.add)

            # ---- tile-0 head + direct f32 gathers (overlap the table
            # build: these need only uv, not btab) ------------------------
            head0 = C0 > 0
            if head0:
                idx0, nwu0, nwv0 = head_tile(pp, 0)
                plo_f = pp.tile([P, 6 * C0], f32, tag="plo_f")
                phi_f = pp.tile([P, 6 * C0], f32, tag="phi_f")
                for k in range(C0):
                    for dst, eoff in ((plo_f, 0), (phi_f, W * 3)):
                        nc.gpsimd.indirect_dma_start(
                            out=dst[:, 6 * k:6 * (k + 1)],
                            out_offset=None,
                            in_=tex_hw,
                            in_offset=bass.IndirectOffsetOnAxis(
                                ap=idx0[:, k:k + 1], axis=0),
                            element_offset=eoff,
                        )

            # ---- Phase 1: build the bf16 u-pair table --------------------
            # Single HBM read per block: convert rows to bf16, get the
            # +1-row copy via a partition-shifted SBUF->SBUF DMA, and the
            # block-boundary row via one cast-DMA from the f32 texture.
            with tc.tile_pool(name="bpool", bufs=2) as bp:
                for blk in range(16):
                    u0 = blk * 128
                    nr = 128 if blk < 15 else 127      # rows this block
                    a_t = bp.tile([P, ROW], f32, tag="arow")
                    nc.sync.dma_start(
                        out=a_t[:nr, :],
                        in_=_ap(tex_flat, u0 * ROW, [[ROW, nr], [1, ROW]]),
                    )
                    ab = bp.tile([P, ROW], bf16, tag="ab")
                    nc.vector.tensor_copy(ab[:nr, :], a_t[:nr, :])
                    a1b = bp.tile([P, ROW], bf16, tag="a1b")
                    # rows u0+1 .. u0+nr-1 via partition shift
                    nc.sync.dma_start(out=a1b[:nr - 1, :], in_=ab[1:nr, :])
                    # boundary row u0+nr from HBM with f32->bf16 cast
                    nc.gpsimd.dma_start(
                        out=a1b[nr - 1:nr, :],
                        in_=_ap(tex_flat, (u0 + nr) * ROW, [[1, 1], [1, ROW]]),
                    )
                    for c in range(2):      # two 1024-column chunks
                        bt = bp.tile([P, 6 * 1024], bf16, tag="bchunk")
                        voff = c * 1024 * 3
                        for (dst_off, src) in ((0, ab), (3, a1b)):
                            nc.vector.tensor_copy(
                                _ap(bt[:], dst_off,
                                    [bt[:].ap[0], [6, 1024], [1, 3]]),
                                _ap(src[:], voff,
                                    [src[:].ap[0], [3, 1024], [1, 3]]),
                            )
                        nc.sync.dma_start(
                            out=_ap(btab[:], u0 * BROW + c * 6 * 1024,
                                    [[BROW, nr], [1, 6 * 1024]]),
                            in_=bt[:nr, :],
                        )

            tc.strict_bb_all_engine_barrier()

            # ---- Phase 2: per-tile sample --------------------------------
            with tc.tile_pool(name="main", bufs=2) as mp:
                for ti in range(ntiles):
                    t0 = head0 and ti == 0
                    if t0:
                        idx, nwu, nwv = idx0, nwu0, nwv0
                        cs = C0        # first C0 columns already gathered
                    else:
                        idx, nwu, nwv = head_tile(mp, ti)
                        cs = 0

                    # gather 24B blocks: [p00, p10, p01, p11] (u-interleaved)
                    patch = mp.tile([P, 12 * K], bf16, tag="patch")
                    for k in range(cs, K):
                        nc.gpsimd.indirect_dma_start(
                            out=patch[:, 12 * k:12 * (k + 1)],
                            out_offset=None,
                            in_=btab[:],
                            in_offset=bass.IndirectOffsetOnAxis(
                                ap=idx[:, k:k + 1], axis=0),
                        )

                    r0 = mp.tile([P, 3 * K], f32, tag="r0")
                    r1 = mp.tile([P, 3 * K], f32, tag="r1")
                    res = mp.tile([P, 3 * K], f32, tag="res")
                    pap = patch[:]
                    n2 = K - cs
                    blend(
                        cs, n2,
                        _ap(pap, 12 * cs + 0, [pap.ap[0], [12, n2], [1, 3]]),
                        _ap(pap, 12 * cs + 6, [pap.ap[0], [12, n2], [1, 3]]),
                        _ap(pap, 12 * cs + 3, [pap.ap[0], [12, n2], [1, 3]]),
                        _ap(pap, 12 * cs + 9, [pap.ap[0], [12, n2], [1, 3]]),
                        nwv, nwu, r0, r1, res)
                    if t0:
                        lo, hi = plo_f[:], phi_f[:]
                        blend(
                            0, C0,
                            _ap(lo, 0, [lo.ap[0], [6, C0], [1, 3]]),
                            _ap(lo, 3, [lo.ap[0], [6, C0], [1, 3]]),
                            _ap(hi, 0, [hi.ap[0], [6, C0], [1, 3]]),
                            _ap(hi, 3, [hi.ap[0], [6, C0], [1, 3]]),
                            nwv, nwu, r0, r1, res)
                    # tanh + store
                    nc.scalar.activation(
                        out=res[:], in_=res[:],
                        func=mybir.ActivationFunctionType.Tanh)
                    nc.sync.dma_start(out=out_t[ti], in_=res[:])

    nc.compile()
    return nc


_NC_CACHE = {}


def _get_nc(npc):
    if npc not in _NC_CACHE:
        _NC_CACHE[npc] = build_nc(npc)
    return _NC_CACHE[npc]


def kernel(uvs, texture):
    uvs = np.ascontiguousarray(uvs, dtype=np.float32)
    texture = np.ascontiguousarray(texture, dtype=np.float32)
    assert uvs.shape == (N_FULL, 2) and texture.shape == (H, W, 3)
    npc = N_FULL // NCORES
    nc = _get_nc(npc)
    in_maps = [
        {"uvs": uvs[c * npc:(c + 1) * npc], "texture": texture}
        for c in range(NCORES)
    ]
    res = run_bass_kernel_spmd(nc, in_maps, core_ids=list(range(NCORES)))
    return np.concatenate([r["out"] for r in res.results], axis=0)


# revision 10
# speedup vs baseline: 1.2919x; 1.2919x over previous
"""DiffTexture bilinear sampling kernel for TRN2 (8 NeuronCores).

Strategy (data-parallel over sample points, texture replicated):
  - Each core handles N/8 = 1,048,576 points in 16 macro-tiles of
    128x512.
  - Phase 1 (per core): build a bf16 u-pair table in DRAM:
      B[u, v] = [T[u,v,:], T[u+1,v,:]]  (6 bf16 = 12B per entry)
    for u in [0, 2046], v in [0, 2047]. Reading entries (u,v0),(u,v0+1)
    as one 24B block yields all four texels of the bilinear footprint.
    Built with dense DMA loads + DVE interleave/convert copies.
  - Phase 2: per 128-point chunk, one indirect DMA gathers each point's
    24B block (idx = u0*2048 + v0). The toolchain consumes exactly one
    index per partition per instruction, so the gather stream is
    8192 Pool instructions/core (~1.3us each) and dominates; everything
    else is arranged to hide under it. uvs are uniform in [-1, 1) so
    u = (x+1)*1023.5 < 2047: floor(u) <= 2046 and the +1 row/col is
    always in range -- no clamping or edge fix-ups needed.
  - Weights (faithful to the reference's quirk: the FLOOR row/col gets
    the frac weight a): weight of the +1 row/col = (1-fr)*[u != floor],
    carried negated: nw = (fr-1)*[u != floor]; blend via
    r = p_lo + nw*(p_lo - p_hi) using stride-0 broadcast APs for the
    channel dim; tanh on ACT.

Floor is built from the DVE round-to-nearest f32->i32 cast plus a
compare fix-up (no floor ALU op on TRN2).
"""

import numpy as np

import concourse.bass as bass
import concourse.bacc as bacc
import concourse.mybir as mybir
from concourse import tile
from concourse.bass_utils import run_bass_kernel_spmd

H = 2048
W = 2048
N_FULL = 8388608
NCORES = 8
P = 128
K = 512                  # points per partition per macro-tile
TPOINTS = P * K          # 65536 points per macro-tile

f32 = mybir.dt.float32
bf16 = mybir.dt.bfloat16
i32 = mybir.dt.int32

# Columns of tile 0 gathered straight from the f32 texture BEFORE the table
# barrier (2 instrs/column instead of 1, but they run while the Pool engine
# would otherwise idle during the table build).
C0 = 160

ROW = W * 3              # texture row, f32 elements (6144)
BROW = W * 6             # pair-table row, bf16 elements (12288)
BROWS = H - 1            # pair-table rows built (2047)


def _ap(t_ap, extra_offset, dims):
    """Build a raw AP on the same tensor as t_ap with given dims."""
    return bass.AP(t_ap.tensor, t_ap.offset + extra_offset, dims)


def build_nc(npc):
    """Build the per-core Bass program for npc points (npc % TPOINTS == 0)."""
    ntiles = npc // TPOINTS
    nc = bacc.Bacc("TRN2", target_bir_lowering=False)

    uvs = nc.dram_tensor("uvs", [npc, 2], f32, kind="ExternalInput")
    texture = nc.dram_tensor("texture", [H, W, 3], f32, kind="ExternalInput")
    out = nc.dram_tensor("out", [npc, 3], f32, kind="ExternalOutput")
    btab = nc.dram_tensor("btab", [BROWS * W, 6], bf16)  # internal, 50MB

    tex_flat = texture[:].rearrange("h w c -> (h w c)")
    tex_hw = texture[:].rearrange("h w c -> (h w) c")
    uvs_t = uvs[:].rearrange("(t p k) c -> t p (k c)", t=ntiles, p=P, k=K)
    out_t = out[:].rearrange("(t p k) c -> t p (k c)", t=ntiles, p=P, k=K)

    with tile.TileContext(nc) as tc:
        with tc.tile_pool(name="persist", bufs=1) as pp:

            def coord(pool, src_ap, name):
                # returns (nw = -(weight of +1 row/col), floor as f32,
                #          plus two scratch tiles now free for reuse)
                cu = pool.tile([P, K], f32, tag=f"{name}_cu")
                nc.vector.tensor_scalar(
                    out=cu[:], in0=src_ap, scalar1=1.0, scalar2=1023.5,
                    op0=mybir.AluOpType.add, op1=mybir.AluOpType.mult)
                ci = pool.tile([P, K], i32, tag=f"{name}_ci")
                nc.vector.tensor_copy(ci[:], cu[:])        # rint -> i32
                rcf = pool.tile([P, K], f32, tag=f"{name}_rcf")
                nc.vector.tensor_copy(rcf[:], ci[:])       # rint as f32
                d = pool.tile([P, K], f32, tag=f"{name}_d")
                nc.vector.tensor_tensor(                    # d = rcf - u
                    out=d[:], in0=rcf[:], in1=cu[:],
                    op=mybir.AluOpType.subtract)
                nc.vector.tensor_scalar(                    # d = (rcf > u)
                    out=d[:], in0=d[:], scalar1=0.0, scalar2=0.0,
                    op0=mybir.AluOpType.max, op1=mybir.AluOpType.not_equal)
                nc.vector.tensor_tensor(                    # rcf = floor
                    out=rcf[:], in0=rcf[:], in1=d[:],
                    op=mybir.AluOpType.subtract)
                fr = pool.tile([P, K], f32, tag=f"{name}_fr")
                nc.vector.tensor_tensor(                    # fr = u - floor
                    out=fr[:], in0=cu[:], in1=rcf[:],
                    op=mybir.AluOpType.subtract)
                nc.vector.tensor_tensor(                    # d = (u != floor)
                    out=d[:], in0=cu[:], in1=rcf[:],
                    op=mybir.AluOpType.not_equal)
                nc.vector.scalar_tensor_tensor(             # cu = (fr-1)*d
                    out=cu[:], in0=fr[:], scalar=1.0, in1=d[:],
                    op0=mybir.AluOpType.subtract,
                    op1=mybir.AluOpType.mult)
                return cu, rcf, fr, ci

            def head_tile(pool, ti):
                # uv -> coords -> idx (+ negated weights) for one tile
                uv = pool.tile([P, 2 * K], f32, tag="uv")
                nc.sync.dma_start(out=uv[:], in_=uvs_t[ti])
                x_ap = _ap(uv[:], 0, [uv[:].ap[0], [2, K]])
                y_ap = _ap(uv[:], 1, [uv[:].ap[0], [2, K]])
                nwu, u0f, fr_u, ci_u = coord(pool, x_ap, "u")
                nwv, v0f, _fv, _cv = coord(pool, y_ap, "v")
                idxf = fr_u
                nc.vector.scalar_tensor_tensor(
                    out=idxf[:], in0=u0f[:], scalar=float(W), in1=v0f[:],
                    op0=mybir.AluOpType.mult, op1=mybir.AluOpType.add)
                idx = ci_u
                nc.vector.tensor_copy(idx[:], idxf[:])
                return idx, nwu, nwv

            def blend(c0, n, p00, p01, p10, p11, nwv, nwu, r0, r1, res):
                # res[cols c0:c0+n] = bilinear blend; APs passed per source.
                # nwv/nwu are [P, K] tiles broadcast over channels with a
                # stride-0 inner dim.
                def v3(t):
                    return _ap(t[:], 3 * c0, [t[:].ap[0], [3, n], [1, 3]])

                def w3(t):
                    return _ap(t[:], c0, [t[:].ap[0], [1, n], [0, 3]])
                # r0 = p00 + nwv*(p00 - p01)   (within row u0)
                nc.vector.tensor_tensor(out=v3(r0), in0=p00, in1=p01,
                                        op=mybir.AluOpType.subtract)
                nc.vector.tensor_tensor(out=v3(r0), in0=v3(r0), in1=w3(nwv),
                                        op=mybir.AluOpType.mult)
                nc.vector.tensor_tensor(out=v3(r0), in0=v3(r0), in1=p00,
                                        op=mybir.AluOpType.add)
                # r1 = p10 + nwv*(p10 - p11)   (within row u0+1)
                nc.vector.tensor_tensor(out=v3(r1), in0=p10, in1=p11,
                                        op=mybir.AluOpType.subtract)
                nc.vector.tensor_tensor(out=v3(r1), in0=v3(r1), in1=w3(nwv),
                                        op=mybir.AluOpType.mult)
                nc.vector.tensor_tensor(out=v3(r1), in0=v3(r1), in1=p10,
                                        op=mybir.AluOpType.add)
                # res = r0 + nwu*(r0 - r1)
                nc.vector.tensor_tensor(out=v3(res), in0=v3(r0), in1=v3(r1),
                                        op=mybir.AluOpType.subtract)
                nc.vector.tensor_tensor(out=v3(res), in0=v3(res),
                                        in1=w3(nwu),
                                        op=mybir.AluOpType.mult)
                nc.vector.tensor_tensor(out=v3(res), in0=v3(res), in1=v3(r0),
                                        op=mybir.AluOpType.add)

            # ---- tile-0 head + direct f32 gathers (overlap the table
            # build: these need only uv, not btab) ------------------------
            head0 = C0 > 0
            if head0:
                idx0, nwu0, nwv0 = head_tile(pp, 0)
                plo_f = pp.tile([P, 6 * C0], f32, tag="plo_f")
                phi_f = pp.tile([P, 6 * C0], f32, tag="phi_f")
                for k in range(C0):
                    for dst, eoff in ((plo_f, 0), (phi_f, W * 3)):
                        nc.gpsimd.indirect_dma_start(
                            out=dst[:, 6 * k:6 * (k + 1)],
                            out_offset=None,
                            in_=tex_hw,
                            in_offset=bass.IndirectOffsetOnAxis(
                                ap=idx0[:, k:k + 1], axis=0),
                            element_offset=eoff,
                        )

            # ---- Phase 1: build the bf16 u-pair table --------------------
            with tc.tile_pool(name="bpool", bufs=2) as bp:
                for blk in range(16):
                    u0 = blk * 128
                    nr = 128 if blk < 15 else 127      # rows this block
                    a_t = bp.tile([P, ROW], f32, tag="arow")
                    a1_t = bp.tile([P, ROW], f32, tag="a1row")
                    nc.sync.dma_start(
                        out=a_t[:nr, :],
                        in_=_ap(tex_flat, u0 * ROW, [[ROW, nr], [1, ROW]]),
                    )
                    nc.sync.dma_start(
                        out=a1_t[:nr, :],
                        in_=_ap(tex_flat, (u0 + 1) * ROW,
                                [[ROW, nr], [1, ROW]]),
                    )
                    for c in range(2):      # two 1024-column chunks
                        bt = bp.tile([P, 6 * 1024], bf16, tag="bchunk")
                        voff = c * 1024 * 3
                        for (dst_off, src) in ((0, a_t), (3, a1_t)):
                            nc.vector.tensor_copy(
                                _ap(bt[:], dst_off,
                                    [bt[:].ap[0], [6, 1024], [1, 3]]),
                                _ap(src[:], voff,
                                    [src[:].ap[0], [3, 1024], [1, 3]]),
                            )
                        nc.sync.dma_start(
                            out=_ap(btab[:], u0 * BROW + c * 6 * 1024,
                                    [[BROW, nr], [1, 6 * 1024]]),
                            in_=bt[:nr, :],
                        )

            tc.strict_bb_all_engine_barrier()

            # ---- Phase 2: per-tile sample --------------------------------
            with tc.tile_pool(name="main", bufs=2) as mp:
                for ti in range(ntiles):
                    t0 = head0 and ti == 0
                    if t0:
                        idx, nwu, nwv = idx0, nwu0, nwv0
                        cs = C0        # first C0 columns already gathered
                    else:
                        idx, nwu, nwv = head_tile(mp, ti)
                        cs = 0

                    # gather 24B blocks: [p00, p10, p01, p11] (u-interleaved)
                    patch = mp.tile([P, 12 * K], bf16, tag="patch")
                    for k in range(cs, K):
                        nc.gpsimd.indirect_dma_start(
                            out=patch[:, 12 * k:12 * (k + 1)],
                            out_offset=None,
                            in_=btab[:],
                            in_offset=bass.IndirectOffsetOnAxis(
                                ap=idx[:, k:k + 1], axis=0),
                        )

                    r0 = mp.tile([P, 3 * K], f32, tag="r0")
                    r1 = mp.tile([P, 3 * K], f32, tag="r1")
                    res = mp.tile([P, 3 * K], f32, tag="res")
                    pap = patch[:]
                    n2 = K - cs
                    blend(
                        cs, n2,
                        _ap(pap, 12 * cs + 0, [pap.ap[0], [12, n2], [1, 3]]),
                        _ap(pap, 12 * cs + 6, [pap.ap[0], [12, n2], [1, 3]]),
                        _ap(pap, 12 * cs + 3, [pap.ap[0], [12, n2], [1, 3]]),
                        _ap(pap, 12 * cs + 9, [pap.ap[0], [12, n2], [1, 3]]),
                        nwv, nwu, r0, r1, res)
                    if t0:
                        lo, hi = plo_f[:], phi_f[:]
                        blend(
                            0, C0,
                            _ap(lo, 0, [lo.ap[0], [6, C0], [1, 3]]),
                            _ap(lo, 3, [lo.ap[0], [6, C0], [1, 3]]),
                            _ap(hi, 0, [hi.ap[0], [6, C0], [1, 3]]),
                            _ap(hi, 3, [hi.ap[0], [6, C0], [1, 3]]),
                            nwv, nwu, r0, r1, res)
                    # tanh + store
                    nc.scalar.activation(
                        out=res[:], in_=res[:],
                        func=mybir.ActivationFunctionType.Tanh)
                    nc.sync.dma_start(out=out_t[ti], in_=res[:])

    nc.compile()
    return nc


_NC_CACHE = {}


def _get_nc(npc):
    if npc not in _NC_CACHE:
        _NC_CACHE[npc] = build_nc(npc)
    return _NC_CACHE[npc]


def kernel(uvs, texture):
    uvs = np.ascontiguousarray(uvs, dtype=np.float32)
    texture = np.ascontiguousarray(texture, dtype=np.float32)
    assert uvs.shape == (N_FULL, 2) and texture.shape == (H, W, 3)
    npc = N_FULL // NCORES
    nc = _get_nc(npc)
    in_maps = [
        {"uvs": uvs[c * npc:(c + 1) * npc], "texture": texture}
        for c in range(NCORES)
    ]
    res = run_bass_kernel_spmd(nc, in_maps, core_ids=list(range(NCORES)))
    return np.concatenate([r["out"] for r in res.results], axis=0)
